# revision 56
# baseline (speedup 1.0000x reference)
"""GQA attention (B=1, L=2048, D=2048, H=32, KV=8, HD=64) + RoPE + causal mask,
tensor-parallel over heads across 8 TRN2 NeuronCores.

Core i owns KV head i and Q heads 4i..4i+3. Each core computes
partial_i = O_i @ wo_i; the host sums the 8 bf16 partials.

Pipeline (per core):
  - x streams in as eight 256-position half-chunks via SWDGE DMA-cast
    (fp32 HBM -> bf16 SBUF in-flight, no engine cast; the host pre-blocks
    x so each partition's half-chunk is one contiguous 16KB descriptor).
    K/V/Q projections run as three single-bank psum passes per half and
    pipeline right behind the stream; a junk-matmul burst bridges the
    initial DMA latency so the PE's HAM clock stays un-throttled.
  - RoPE runs on DVE in bf16 (ACT evacuates the projection psum to bf16
    first, so the multiplies hit the 2x packed mode); the [cos;sin] /
    [sin;cos] stacked tables make each head-pair 4 mults + 4 combines
    with verifier-legal matched input partitions.
  - The V stationary carries 64 ones-columns, so the softmax denominator
    lands replicated on PSUM partitions 64:128 - normalization is one
    reciprocal + two multiplies on DVE, no partition broadcast.
  - Attention runs in 512-query stripes; the two heads of a pair issue
    back-to-back K=64 matmuls (row-tiled halves of the PE array), one
    fused Exp covers both heads, and V matmuls accumulate O^T + denom.
    Causal masking: block-granular wedge trim plus a -1e9 PE matmul on
    diagonal blocks. Stripe 0 runs inside the x stream; stripes 1-2 drip
    between projection matmuls; stripe-2 pb1 + stripe 3 + the wo matmuls
    fill phase B, outputs streaming to HBM per 128-query block.
"""

import numpy as np

try:
    import concourse  # noqa: F401
except ImportError:
    import sys as _sys
    for _p in ("/opt/trn_rl_repo", "/root/.axon_site/_ro/trn_rl_repo"):
        if _p not in _sys.path:
            _sys.path.insert(0, _p)

B, L, D = 1, 2048, 2048
H, KV, HD = 32, 8, 64
NCORES = 8
P = 128
KT = D // P          # 16 contraction tiles
LB = L // P          # 16 key blocks
NCH = 4              # x position chunks of 512

USE_SWDGE = True     # cast fp32->bf16 during DMA (SWDGE) vs ACT cast


def _build_nc():
    import concourse.mybir as mybir
    import concourse.tile as tile
    from concourse import bacc
    from concourse.bass import ts, ds
    from concourse.masks import make_identity

    f32 = mybir.dt.float32
    bf16 = mybir.dt.bfloat16
    EXP = mybir.ActivationFunctionType.Exp
    ADD = mybir.AluOpType.add
    SUB = mybir.AluOpType.subtract
    MULT = mybir.AluOpType.mult

    nc = bacc.Bacc(None, target_bir_lowering=False, debug=False)

    # x re-blocked host-side: row 128*hc+p holds positions 256*hc..+256 for
    # all 16 contraction tiles contiguously -> one 16KB descriptor per
    # partition per half-chunk (vs 2KB strided runs from a flat [D, L]).
    x_b = nc.declare_dram_parameter("x_b", [8 * P, KT * 256], f32,
                                    isOutput=False)
    wq_p = nc.declare_dram_parameter("wq_p", [P, KT * 256], f32, isOutput=False)
    wkv = nc.declare_dram_parameter("wkv", [P, KT * 128], f32, isOutput=False)
    wo_p = nc.declare_dram_parameter("wo_p", [256, D], f32, isOutput=False)
    cos2 = nc.declare_dram_parameter("cos2", [64, L], f32, isOutput=False)
    sin2 = nc.declare_dram_parameter("sin2", [64, L], f32, isOutput=False)
    part = nc.declare_dram_parameter("part", [L, D], bf16, isOutput=True)

    with tile.TileContext(nc) as tc:
        with tc.tile_pool(name="persist", bufs=1) as pp:
            # ---------- persistent SBUF ----------
            wq_sb = pp.tile([P, KT, 256], bf16, tag="wq")
            kv_sb = pp.tile([P, KT, 128], bf16, tag="kv")
            wo_sb = pp.tile([P, 2, L], bf16, tag="wo")
            qt_sb = pp.tile([P, 2, L], bf16, tag="qt")   # [A(re,im)|B(re,im)]
            kt_sb = pp.tile([P, L], bf16, tag="kt")      # rows 64:128 dup
            vt_sb = pp.tile([64, L], bf16, tag="vt")     # V^T [dd, k]
            vn_sb = pp.tile([P, LB, P], bf16, tag="vn")  # cols 64:128 = ones
            ot_sb = pp.tile([P, 2, L], bf16, tag="ot")
            t1_sb = pp.tile([P, L], bf16, tag="t1")      # [cos2; sin2]
            t2_sb = pp.tile([P, L], bf16, tag="t2")      # [sin2; cos2]
            ident = pp.tile([64, 64], bf16, tag="ident")
            negi = pp.tile([P, P], bf16, tag="negi")     # -1e9 on the diagonal
            ustr = pp.tile([P, P], bf16, tag="ustr")     # 1 where k > q
            warm = pp.tile([1, 16], f32, tag="warm")

            # preload the ACT exp table off the critical path
            nc.vector.memset(warm[:], 0.0)
            nc.scalar.activation(warm[:], warm[:], EXP)
            wsrc = pp.tile([64, 512], bf16, tag="wsrc")
            nc.vector.memset(wsrc[:], 0.0)

            # ---------- input DMA (SWDGE queue: priority order) ----------
            def in_cast_dma(out_ap, in_ap):
                nc.gpsimd.dma_start(out_ap, in_ap)

            with (
                tc.tile_pool(name="xt_p", bufs=3) as xtp,
                tc.tile_pool(name="qraw_p", bufs=2) as qrp,
                tc.tile_pool(name="kraw_p", bufs=2) as krp,
                tc.tile_pool(name="m_p", bufs=2) as mp,
                tc.tile_pool(name="et_p", bufs=4) as etp,
                tc.tile_pool(name="rb_p", bufs=2) as rbp,
                tc.tile_pool(name="ob_p", bufs=4) as obp,
            ):
                def half_dma(xt, c, h):
                    in_cast_dma(
                        xt[:, h, :, :].rearrange("p t l -> p (t l)"),
                        x_b[ds(P * (2 * c + h), P), :])

                def x_chunk_dma(c, half=None):
                    xt = xtp.tile([P, 2, KT, 256], bf16, tag="xt_c",
                                  name=f"xt{c}")
                    half_dma(xt, c, 0)
                    half_dma(xt, c, 1)
                    return xt

                # SWDGE ring priority order: x0 halves + weights, then x1-x3
                xt0 = xtp.tile([P, 2, KT, 256], bf16, tag="xt_c", name="xt0")
                half_dma(xt0, 0, 0)
                in_cast_dma(
                    kv_sb[:, :, :].rearrange("p t n -> p (t n)"), wkv[:, :])
                half_dma(xt0, 0, 1)
                in_cast_dma(
                    wq_sb[:, :, :].rearrange("p t n -> p (t n)"), wq_p[:, :])
                xts = [xt0, x_chunk_dma(1)]
                # cos/sin via the idle HWDGE ring (f32); ACT builds t1/t2
                cssn = pp.tile([P, L], f32, tag="cssn")
                nc.sync.dma_start(cssn[0:64, :], cos2[:, :])
                nc.sync.dma_start(cssn[64:128, :], sin2[:, :])
                nc.scalar.copy(t1_sb[:, :], cssn[:, :])
                nc.scalar.copy(t2_sb[0:64, :], cssn[64:128, :])
                nc.scalar.copy(t2_sb[64:128, :], cssn[0:64, :])

                # preamble constants (gpsimd engine, after DMA issues)
                make_identity(nc, ident[:])
                nc.gpsimd.memset(negi[:], 0.0)
                nc.gpsimd.affine_select(
                    out=negi[:], in_=negi[:],
                    compare_op=mybir.AluOpType.not_equal, fill=-1e9,
                    base=0, channel_multiplier=1, pattern=[[-1, P]],
                )
                nc.gpsimd.memset(ustr[:], 1.0)
                nc.gpsimd.affine_select(
                    out=ustr[:], in_=ustr[:],
                    compare_op=mybir.AluOpType.is_gt, fill=0.0,
                    base=0, channel_multiplier=1, pattern=[[-1, P]],
                )
                nc.vector.memset(vn_sb[:, :, 64:128], 1.0)

                drip = []

                def pop_drip(k=1):
                    for _ in range(k):
                        if drip:
                            drip.pop(0)()

                # ---------- phase A ----------
                # 3-pass projection through a 1-bank psum pool (bufs=2)
                # leaves 6 banks for the dripped attention stripes.
                with (
                    tc.tile_pool(name="ps_pj", bufs=2, space="PSUM") as ps_pj,
                    tc.tile_pool(name="ps_sA", bufs=2, space="PSUM") as ps_sA,
                    tc.tile_pool(name="ps_oA", bufs=1, space="PSUM") as ps_oA,
                ):
                    def proj_chunk(c, xt, rate=0, pass_major=False):
                        ch0 = 512 * c
                        w = 256
                        kraw = krp.tile([64, 512], bf16, tag="kraw",
                                        name=f"kraw{c}")
                        qraw = qrp.tile([P, 2, 512], bf16, tag="qraw",
                                        name=f"qraw{c}")

                        def one_pass(pi, h):
                            hl = ds(h * w, w)
                            if pi == 0:
                                stat = kv_sb[:, :, :]

                                def evac(pps):
                                    nc.scalar.copy(kraw[:, hl],
                                                   pps[0:64, :])
                                    nc.scalar.copy(
                                        vt_sb[:, ds(ch0 + h * w, w)],
                                        pps[64:128, :])
                            else:
                                pb = pi - 1
                                stat = wq_sb[:, :, ts(pb, P)]

                                def evac(pps, pb=pb):
                                    nc.scalar.copy(qraw[:, pb, hl],
                                                   pps[:, :])
                            pps = ps_pj.tile([P, 512], f32, tag="pj_ps",
                                             name=f"pj{c}_{h}_{pi}")
                            for t in range(KT):
                                nc.tensor.matmul(
                                    pps[:, 0:w], stat[:, t, :],
                                    xt[:, h, t, :],
                                    start=(t == 0), stop=(t == KT - 1))
                                if rate and t % rate == rate - 1:
                                    pop_drip(1)
                            evac(pps[:, 0:w])

                        if pass_major:
                            for pi in range(3):
                                for h in range(2):
                                    one_pass(pi, h)
                        else:
                            for h in range(2):
                                for pi in range(3):
                                    one_pass(pi, h)
                        return kraw, qraw

                    def rope_chunk(c, kraw, qraw):
                        # tensor_tensor inputs must share a partition range;
                        # products land in tiles re-based at partition 0 so
                        # the combines read matched ranges.
                        ch = ds(512 * c, 512)
                        mka = mp.tile([32, 2, 512], bf16, tag="mka",
                                      name=f"mka{c}")
                        mkb = mp.tile([32, 2, 512], bf16, tag="mkb",
                                      name=f"mkb{c}")
                        # [kre*c, kre*s] and [kim*s, kim*c]
                        nc.vector.tensor_tensor(
                            mka[:, 0, :], kraw[0:32, :], t1_sb[0:32, ch], MULT)
                        nc.vector.tensor_tensor(
                            mka[:, 1, :], kraw[0:32, :], t2_sb[0:32, ch], MULT)
                        nc.vector.tensor_tensor(
                            mkb[:, 0, :], kraw[32:64, :], t2_sb[32:64, ch],
                            MULT)
                        nc.vector.tensor_tensor(
                            mkb[:, 1, :], kraw[32:64, :], t1_sb[32:64, ch],
                            MULT)
                        nc.vector.tensor_tensor(
                            kt_sb[0:32, ch], mka[:, 0, :], mkb[:, 0, :], SUB)
                        nc.vector.tensor_tensor(
                            kt_sb[32:64, ch], mka[:, 1, :], mkb[:, 1, :], ADD)
                        # dup K rows for the row-tiled head pair
                        nc.sync.dma_start(kt_sb[64:128, ch], kt_sb[0:64, ch])
                        for pb in range(2):
                            mqa = mp.tile([64, 2, 512], bf16, tag="mqa",
                                          name=f"mqa{c}_{pb}")
                            mqb = mp.tile([64, 2, 512], bf16, tag="mqb",
                                          name=f"mqb{c}_{pb}")
                            # re products: [ABre*c, ABre*s]
                            nc.vector.tensor_tensor(
                                mqa[:, 0, :], qraw[0:64, pb, :],
                                t1_sb[0:64, ch], MULT)
                            nc.vector.tensor_tensor(
                                mqa[:, 1, :], qraw[0:64, pb, :],
                                t2_sb[0:64, ch], MULT)
                            # im products: [ABim*s, ABim*c]
                            nc.vector.tensor_tensor(
                                mqb[:, 0, :], qraw[64:128, pb, :],
                                t1_sb[64:128, ch], MULT)
                            nc.vector.tensor_tensor(
                                mqb[:, 1, :], qraw[64:128, pb, :],
                                t2_sb[64:128, ch], MULT)
                            nc.vector.tensor_tensor(
                                qt_sb[0:32, pb, ch],
                                mqa[0:32, 0, :], mqb[0:32, 0, :], SUB)
                            nc.vector.tensor_tensor(
                                qt_sb[64:96, pb, ch],
                                mqa[32:64, 0, :], mqb[32:64, 0, :], SUB)
                            nc.vector.tensor_tensor(
                                qt_sb[32:64, pb, ch],
                                mqa[0:32, 1, :], mqb[0:32, 1, :], ADD)
                            nc.vector.tensor_tensor(
                                qt_sb[96:128, pb, ch],
                                mqa[32:64, 1, :], mqb[32:64, 1, :], ADD)

                    def vt_unit(c):
                        vps = ps_sA.tile([P, 4, 64], bf16, tag="s_ps",
                                         name=f"vps{c}")
                        for j in range(4):
                            nc.tensor.matmul(
                                vps[:, j, :], vt_sb[:, ts(4 * c + j, P)],
                                ident[:], start=True, stop=True,
                                is_transpose=True)
                        nc.vector.tensor_copy(
                            vn_sb[:, ds(4 * c, 4), 0:64], vps[:, :, :])

                    def stripe_units(s, ps_s, ps_o, pbs=(0, 1)):
                        qs = 512 * s
                        nkb = 4 * (s + 1)
                        units = []
                        for pb in pbs:
                            st = {}

                            def mk_unit(pb=pb, kb=0, st=st):
                                def unit():
                                    if "ops" not in st:
                                        st["ops"] = ps_o.tile(
                                            [P, 2, 512], f32, tag="o_ps",
                                            name=f"ops{s}_{pb}")
                                    ops = st["ops"]
                                    c0 = max(0, kb - 4 * s) * P
                                    w = 512 - c0
                                    diag = kb >= 4 * s
                                    sps = ps_s.tile([P, 2, 512], f32,
                                                    tag="s_ps",
                                                    name=f"sps{s}_{pb}_{kb}")
                                    for hh in range(2):
                                        nc.tensor.matmul(
                                            sps[:, hh, c0:512],
                                            kt_sb[ds(64 * hh, 64), ts(kb, P)],
                                            qt_sb[ds(64 * hh, 64), pb,
                                                  ds(qs + c0, w)],
                                            start=True, stop=not diag)
                                    if diag:
                                        for hh in range(2):
                                            nc.tensor.matmul(
                                                sps[:, hh, c0:c0 + P],
                                                negi[:], ustr[:],
                                                start=False, stop=True)
                                    et = etp.tile([P, 2, 512], bf16,
                                                  tag="e_t")
                                    if c0 == 0:
                                        nc.scalar.activation(
                                            et[:, :, :].rearrange(
                                                "p a b -> p (a b)"),
                                            sps[:, :, :].rearrange(
                                                "p a b -> p (a b)"),
                                            EXP, scale=0.125)
                                    else:
                                        nc.scalar.activation(
                                            et[:, :, c0:512],
                                            sps[:, :, c0:512],
                                            EXP, scale=0.125)
                                    # software-pipeline: defer this unit's AV
                                    # to the next unit so the exp's ~1.15us
                                    # ACT latency hides behind the next S
                                    # matmuls + wo drip instead of stalling
                                    # the PE.
                                    def av(kb=kb, et=et, c0=c0, w=w):
                                        for hh in range(2):
                                            nc.tensor.matmul(
                                                ops[:, hh, c0:512],
                                                vn_sb[:, kb, :],
                                                et[:, hh, ds(c0, w)],
                                                start=(kb == 0),
                                                stop=(kb == nkb - 1))
                                    prev = st.pop("pending", None)
                                    if prev is not None:
                                        prev()
                                    if kb != nkb - 1:
                                        st["pending"] = av
                                    if kb == nkb - 1:
                                        av()
                                        # split the last stripe's norm into
                                        # q-halves so the tail wo can start
                                        # after the first half
                                        nh = 2 if (s == 3 and pb == 1) else 1
                                        wq_ = 512 // nh
                                        for qh in range(nh):
                                            qsl = ds(qh * wq_, wq_)
                                            dn = rbp.tile(
                                                [64, 2, wq_], f32, tag="dn",
                                                name=f"dn{s}_{pb}_{qh}")
                                            rb = rbp.tile(
                                                [64, 2, wq_], f32, tag="rb",
                                                name=f"rb{s}_{pb}_{qh}")
                                            # custom-DVE recip needs SBUF in
                                            nc.vector.tensor_copy(
                                                dn[:, :, :],
                                                ops[64:128, :, qsl])
                                            nc.vector.reciprocal_approx_fast(
                                                rb[:, :, :].rearrange(
                                                    "p a b -> p (a b)"),
                                                dn[:, :, :].rearrange(
                                                    "p a b -> p (a b)"))
                                            for hh in range(2):
                                                nc.vector.tensor_tensor(
                                                    ot_sb[ds(64 * hh, 64), pb,
                                                          ds(qs + qh * wq_,
                                                             wq_)],
                                                    ops[0:64, hh, qsl],
                                                    rb[:, hh, :], MULT)
                                return unit

                            for kb in range(nkb):
                                units.append(mk_unit(pb=pb, kb=kb, st=st))
                        return units

                    # warm the PE while the first x bytes stream in: HAM
                    # un-throttles after ~3.4us of sustained matmul activity
                    # and re-throttles after ~3.4us idle, so keep it busy
                    # until the real stream begins (~20us in).
                    junk_n = [0]

                    def junk(k):
                        # junk matmuls keep HAM un-throttled across head
                        # gaps where no real PE work is ready yet
                        jt = ps_sA.tile([64, 512], f32, tag="s_ps",
                                        name=f"junk{junk_n[0]}")
                        junk_n[0] += 1
                        for _ in range(k):
                            nc.tensor.matmul(
                                jt[:, :], wsrc[:, 0:64], wsrc[:, :],
                                start=True, stop=True)

                    junk(24)
                    # chunk 0 (two position-halves for early start,
                    # pass-major so each pass starts as its input lands)
                    kraw0, qraw0 = proj_chunk(0, xts[0], pass_major=True)
                    xts.append(x_chunk_dma(2))
                    rope_chunk(0, kraw0, qraw0)
                    vt_unit(0)
                    # stripe 0 runs directly: it covers the xt1 DMA wait
                    # (all of its inputs come from chunk 0)
                    for u in stripe_units(0, ps_sA, ps_oA):
                        u()
                    # chunk 1 (stripe 1 needs rope1 - no drip yet)
                    kraw1, qraw1 = proj_chunk(1, xts[1])
                    xts.append(x_chunk_dma(3))
                    rope_chunk(1, kraw1, qraw1)
                    vt_unit(1)
                    # chunk 2, stripe 1 dripped in
                    drip += stripe_units(1, ps_sA, ps_oA)
                    kraw2, qraw2 = proj_chunk(2, xts[2], rate=5)
                    rope_chunk(2, kraw2, qraw2)
                    vt_unit(2)
                    # chunk 3, stripe-2 pb0 dripped in (pb1 goes to phase B
                    # so the A->B psum-pool barrier comes sooner)
                    drip += stripe_units(2, ps_sA, ps_oA, pbs=(0,))
                    kraw3, qraw3 = proj_chunk(3, xts[3], rate=8)
                    # wo weights (queued after x on the SWDGE ring)
                    in_cast_dma(
                        wo_sb[:, :, :],
                        wo_p[:, :].rearrange("(c p) d -> p c d", p=P))
                    rope_chunk(3, kraw3, qraw3)
                    vt_unit(3)
                    while drip:
                        pop_drip()

                    ob_state = {}

                    def wo_unit(lq, n, hf, evac_act=None, pool=None,
                                ptag="w_ps"):
                        def unit():
                            key = (lq, n)
                            if key not in ob_state:
                                ob_state[key] = obp.tile(
                                    [P, 1024], bf16, tag="o_sb",
                                    name=f"ob{lq}_{n}")
                            ob = ob_state[key]
                            wps = pool.tile([P, 512], f32, tag=ptag,
                                            name=f"wps{lq}_{n}_{hf}")
                            for t in range(2):
                                nc.tensor.matmul(
                                    wps[:, :], ot_sb[:, t, ts(lq, P)],
                                    wo_sb[:, t, ds(n * 1024 + hf * 512, 512)],
                                    start=(t == 0), stop=(t == 1))
                            use_act = evac_act
                            if use_act is None:
                                use_act = False
                            if use_act:
                                nc.scalar.copy(
                                    ob[:, ds(512 * hf, 512)], wps[:, :])
                            else:
                                nc.vector.tensor_copy(
                                    ob[:, ds(512 * hf, 512)], wps[:, :])
                            # stream each half out as soon as it lands
                            deng = nc.sync if (lq + n) % 2 == 0 \
                                else nc.gpsimd
                            deng.dma_start(
                                part[ts(lq, P), ds(n * 1024 + hf * 512, 512)],
                                ob[:, ds(512 * hf, 512)])
                        return unit

                # ---------- phase B: stripes 2-3 + wo ----------
                with (
                    tc.tile_pool(name="ps_sB", bufs=2, space="PSUM") as ps_sB,
                    tc.tile_pool(name="ps_oB", bufs=1, space="PSUM") as ps_oB,
                    tc.tile_pool(name="ps_w", bufs=2, space="PSUM") as ps_w,
                ):
                    drip += [wo_unit(lq, n, hf, pool=ps_w)
                             for lq in range(0, 12)
                             for n in range(2) for hf in range(2)]
                    for u in stripe_units(2, ps_sB, ps_oB, pbs=(1,)):
                        u()
                        pop_drip(1)
                    for u in stripe_units(3, ps_sB, ps_oB):
                        u()
                        pop_drip(2)
                    while drip:
                        pop_drip()
                    for lq in range(12, 16):
                        for n in range(2):
                            for hf in range(2):
                                wo_unit(lq, n, hf, evac_act=(hf == 0),
                                        pool=ps_w)()

    nc.compile()
    return nc


_NC_CACHE = None


def _get_nc():
    global _NC_CACHE
    if _NC_CACHE is None:
        _NC_CACHE = _build_nc()
    return _NC_CACHE


def _shard_inputs(x, wq, wk, wv, wo, freqs_cos, freqs_sin, mask):
    """Host-side shard prep: pure layout/indexing transforms, no arithmetic."""
    f = np.float32
    perm = np.empty(64, np.int64)
    perm[:32] = 2 * np.arange(32)
    perm[32:] = 2 * np.arange(32) + 1

    xr = np.asarray(x, f).reshape(L, D)
    x_b = np.ascontiguousarray(
        xr.reshape(8, 256, KT, P).transpose(0, 3, 2, 1)
        .reshape(8 * P, KT * 256))
    cosT = np.ascontiguousarray(np.asarray(freqs_cos, f).T)
    sinT = np.ascontiguousarray(np.asarray(freqs_sin, f).T)
    cos2 = np.ascontiguousarray(np.concatenate([cosT, cosT], 0))
    sin2 = np.ascontiguousarray(np.concatenate([sinT, sinT], 0))

    wq = np.asarray(wq, f)
    wk = np.asarray(wk, f)
    wv = np.asarray(wv, f)
    wo = np.asarray(wo, f)

    in_maps = []
    for i in range(NCORES):
        wq_i = wq[:, 4 * i * 64:(4 * i + 4) * 64]
        cols = []
        for pb in range(2):
            A = wq_i[:, (2 * pb) * 64:(2 * pb + 1) * 64][:, perm]
            Bc = wq_i[:, (2 * pb + 1) * 64:(2 * pb + 2) * 64][:, perm]
            cols.append(np.concatenate(
                [A[:, :32], Bc[:, :32], A[:, 32:], Bc[:, 32:]], 1))
        wq_p = np.concatenate(cols, 1)
        # device layout: [p, t*n] with row 128t+p -> partition p, chunk t
        wq_p = np.ascontiguousarray(
            wq_p.reshape(KT, P, 256).transpose(1, 0, 2).reshape(P, KT * 256))
        wk_p = wk[:, i * 64:(i + 1) * 64][:, perm]
        wv_i = wv[:, i * 64:(i + 1) * 64]
        wkv = np.concatenate([wk_p, wv_i], 1)
        wkv = np.ascontiguousarray(
            wkv.reshape(KT, P, 128).transpose(1, 0, 2).reshape(P, KT * 128))
        wo_i = np.ascontiguousarray(wo[4 * i * 64:(4 * i + 4) * 64, :])
        in_maps.append({
            "x_b": x_b, "wq_p": wq_p, "wkv": wkv, "wo_p": wo_i,
            "cos2": cos2, "sin2": sin2,
        })
    return in_maps


_last_results = None


def kernel(x, wq, wk, wv, wo, freqs_cos, freqs_sin, mask):
    global _last_results
    from concourse.bass_utils import run_bass_kernel_spmd

    nc = _get_nc()
    in_maps = _shard_inputs(x, wq, wk, wv, wo, freqs_cos, freqs_sin, mask)
    res = run_bass_kernel_spmd(nc, in_maps, core_ids=list(range(NCORES)))
    _last_results = res
    out = np.zeros((L, D), np.float32)
    for i in range(NCORES):
        out += np.asarray(res.results[i]["part"]).astype(np.float32)
    return out.reshape(B, L, D)


# revision 57
# speedup vs baseline: 1.0354x; 1.0354x over previous
"""GQA attention (B=1, L=2048, D=2048, H=32, KV=8, HD=64) + RoPE + causal mask,
tensor-parallel over heads across 8 TRN2 NeuronCores.

Core i owns KV head i and Q heads 4i..4i+3. Each core computes
partial_i = O_i @ wo_i; the host sums the 8 bf16 partials.

Pipeline (per core):
  - x^T streams in as four 512-position chunks (all 16 contraction tiles
    per chunk), so K/V/Q projections, RoPE, and the first two attention
    stripes all pipeline behind the DMA instead of waiting for the full x.
  - RoPE runs on DVE in bf16 (psum evacuated to bf16 by ACT first, so the
    multiplies hit the 2x packed mode); cos/sin tables are stacked
    [cos;sin] / [sin;cos] so each head-pair needs 2 mults + 4 combines.
  - The V stationary carries 64 ones-columns, so the softmax denominator
    lands replicated on PSUM partitions 64:128 - normalization is one
    reciprocal + two multiplies on DVE, no partition broadcast.
  - Attention runs in 512-query stripes; the two heads of a pair issue
    back-to-back K=64 matmuls (row-tiled halves of the PE array), one
    fused Exp covers both heads, and V matmuls accumulate O^T + denom.
    Causal masking: block-granular wedge trim plus a -1e9 PE matmul on
    diagonal blocks.
  - wo matmuls drip between attention matmuls; outputs stream out as
    they complete.
"""

import numpy as np

try:
    import concourse  # noqa: F401
except ImportError:
    import sys as _sys
    for _p in ("/opt/trn_rl_repo", "/root/.axon_site/_ro/trn_rl_repo"):
        if _p not in _sys.path:
            _sys.path.insert(0, _p)

B, L, D = 1, 2048, 2048
H, KV, HD = 32, 8, 64
NCORES = 8
P = 128
KT = D // P          # 16 contraction tiles
LB = L // P          # 16 key blocks
NCH = 4              # x position chunks of 512

USE_SWDGE = True     # cast fp32->bf16 during DMA (SWDGE) vs ACT cast


def _build_nc():
    import concourse.mybir as mybir
    import concourse.tile as tile
    from concourse import bacc
    from concourse.bass import ts, ds
    from concourse.masks import make_identity

    f32 = mybir.dt.float32
    bf16 = mybir.dt.bfloat16
    EXP = mybir.ActivationFunctionType.Exp
    ADD = mybir.AluOpType.add
    SUB = mybir.AluOpType.subtract
    MULT = mybir.AluOpType.mult

    nc = bacc.Bacc(None, target_bir_lowering=False, debug=False)

    # x re-blocked host-side: row 128*hc+p holds positions 256*hc..+256 for
    # all 16 contraction tiles contiguously -> one 16KB descriptor per
    # partition per half-chunk (vs 2KB strided runs from a flat [D, L]).
    x_b = nc.declare_dram_parameter("x_b", [8 * P, KT * 256], f32,
                                    isOutput=False)
    wq_p = nc.declare_dram_parameter("wq_p", [P, KT * 256], f32, isOutput=False)
    wkv = nc.declare_dram_parameter("wkv", [P, KT * 128], f32, isOutput=False)
    wo_p = nc.declare_dram_parameter("wo_p", [256, D], f32, isOutput=False)
    cos2 = nc.declare_dram_parameter("cos2", [64, L], f32, isOutput=False)
    sin2 = nc.declare_dram_parameter("sin2", [64, L], f32, isOutput=False)
    part = nc.declare_dram_parameter("part", [L, D], bf16, isOutput=True)

    with tile.TileContext(nc) as tc:
        with tc.tile_pool(name="persist", bufs=1) as pp:
            # ---------- persistent SBUF ----------
            wq_sb = pp.tile([P, KT, 256], bf16, tag="wq")
            kv_sb = pp.tile([P, KT, 128], bf16, tag="kv")
            wo_sb = pp.tile([P, 2, L], bf16, tag="wo")
            qt_sb = pp.tile([P, 2, L], bf16, tag="qt")   # [A(re,im)|B(re,im)]
            kt_sb = pp.tile([P, L], bf16, tag="kt")      # rows 64:128 dup
            vt_sb = pp.tile([64, L], bf16, tag="vt")     # V^T [dd, k]
            vn_sb = pp.tile([P, LB, P], bf16, tag="vn")  # cols 64:128 = ones
            ot_sb = pp.tile([P, 2, L], bf16, tag="ot")
            t1_sb = pp.tile([P, L], bf16, tag="t1")      # [cos2; sin2]
            t2_sb = pp.tile([P, L], bf16, tag="t2")      # [sin2; cos2]
            ident = pp.tile([64, 64], bf16, tag="ident")
            negi = pp.tile([P, P], bf16, tag="negi")     # -1e9 on the diagonal
            ustr = pp.tile([P, P], bf16, tag="ustr")     # 1 where k > q
            warm = pp.tile([1, 16], f32, tag="warm")

            # preload the ACT exp table off the critical path
            nc.vector.memset(warm[:], 0.0)
            nc.scalar.activation(warm[:], warm[:], EXP)
            wsrc = pp.tile([64, 512], bf16, tag="wsrc")
            nc.vector.memset(wsrc[:], 0.0)

            # ---------- input DMA (SWDGE queue: priority order) ----------
            def in_cast_dma(out_ap, in_ap):
                nc.gpsimd.dma_start(out_ap, in_ap)

            with (
                tc.tile_pool(name="xt_p", bufs=3) as xtp,
                tc.tile_pool(name="qraw_p", bufs=2) as qrp,
                tc.tile_pool(name="kraw_p", bufs=2) as krp,
                tc.tile_pool(name="m_p", bufs=2) as mp,
                tc.tile_pool(name="et_p", bufs=4) as etp,
                tc.tile_pool(name="rb_p", bufs=2) as rbp,
                tc.tile_pool(name="ob_p", bufs=4) as obp,
            ):
                def half_dma(xt, c, h):
                    in_cast_dma(
                        xt[:, h, :, :].rearrange("p t l -> p (t l)"),
                        x_b[ds(P * (2 * c + h), P), :])

                def x_chunk_dma(c, half=None):
                    xt = xtp.tile([P, 2, KT, 256], bf16, tag="xt_c",
                                  name=f"xt{c}")
                    half_dma(xt, c, 0)
                    half_dma(xt, c, 1)
                    return xt

                # SWDGE ring priority order: x0 halves + weights, then x1-x3
                xt0 = xtp.tile([P, 2, KT, 256], bf16, tag="xt_c", name="xt0")
                half_dma(xt0, 0, 0)
                in_cast_dma(
                    kv_sb[:, :, :].rearrange("p t n -> p (t n)"), wkv[:, :])
                half_dma(xt0, 0, 1)
                in_cast_dma(
                    wq_sb[:, :, :].rearrange("p t n -> p (t n)"), wq_p[:, :])
                xts = [xt0, x_chunk_dma(1)]
                # cos/sin via the idle HWDGE ring (f32); ACT builds t1/t2
                cssn = pp.tile([P, L], f32, tag="cssn")
                nc.sync.dma_start(cssn[0:64, :], cos2[:, :])
                nc.sync.dma_start(cssn[64:128, :], sin2[:, :])
                nc.scalar.copy(t1_sb[:, :], cssn[:, :])
                nc.scalar.copy(t2_sb[0:64, :], cssn[64:128, :])
                nc.scalar.copy(t2_sb[64:128, :], cssn[0:64, :])

                # preamble constants (gpsimd engine, after DMA issues)
                make_identity(nc, ident[:])
                nc.gpsimd.memset(negi[:], 0.0)
                nc.gpsimd.affine_select(
                    out=negi[:], in_=negi[:],
                    compare_op=mybir.AluOpType.not_equal, fill=-1e9,
                    base=0, channel_multiplier=1, pattern=[[-1, P]],
                )
                nc.gpsimd.memset(ustr[:], 1.0)
                nc.gpsimd.affine_select(
                    out=ustr[:], in_=ustr[:],
                    compare_op=mybir.AluOpType.is_gt, fill=0.0,
                    base=0, channel_multiplier=1, pattern=[[-1, P]],
                )
                nc.vector.memset(vn_sb[:, :, 64:128], 1.0)

                drip = []

                def pop_drip(k=1):
                    for _ in range(k):
                        if drip:
                            drip.pop(0)()

                # ---------- phase A ----------
                # 3-pass projection through a 1-bank psum pool (bufs=2)
                # leaves 6 banks for the dripped attention stripes.
                with (
                    tc.tile_pool(name="ps_pj", bufs=2, space="PSUM") as ps_pj,
                    tc.tile_pool(name="ps_sA", bufs=2, space="PSUM") as ps_sA,
                    tc.tile_pool(name="ps_oA", bufs=1, space="PSUM") as ps_oA,
                ):
                    def proj_chunk(c, xt, rate=0, pass_major=False):
                        ch0 = 512 * c
                        w = 256
                        kraw = krp.tile([64, 512], bf16, tag="kraw",
                                        name=f"kraw{c}")
                        qraw = qrp.tile([P, 2, 512], bf16, tag="qraw",
                                        name=f"qraw{c}")

                        def one_pass(pi, h):
                            hl = ds(h * w, w)
                            if pi == 0:
                                stat = kv_sb[:, :, :]

                                def evac(pps):
                                    nc.scalar.copy(kraw[:, hl],
                                                   pps[0:64, :])
                                    nc.scalar.copy(
                                        vt_sb[:, ds(ch0 + h * w, w)],
                                        pps[64:128, :])
                            else:
                                pb = pi - 1
                                stat = wq_sb[:, :, ts(pb, P)]

                                def evac(pps, pb=pb):
                                    nc.scalar.copy(qraw[:, pb, hl],
                                                   pps[:, :])
                            pps = ps_pj.tile([P, 512], f32, tag="pj_ps",
                                             name=f"pj{c}_{h}_{pi}")
                            for t in range(KT):
                                nc.tensor.matmul(
                                    pps[:, 0:w], stat[:, t, :],
                                    xt[:, h, t, :],
                                    start=(t == 0), stop=(t == KT - 1))
                                if rate and t % rate == rate - 1:
                                    pop_drip(1)
                            evac(pps[:, 0:w])

                        if pass_major:
                            for pi in range(3):
                                for h in range(2):
                                    one_pass(pi, h)
                        else:
                            for h in range(2):
                                for pi in range(3):
                                    one_pass(pi, h)
                        return kraw, qraw

                    def rope_chunk(c, kraw, qraw):
                        # tensor_tensor inputs must share a partition range;
                        # products land in tiles re-based at partition 0 so
                        # the combines read matched ranges.
                        ch = ds(512 * c, 512)
                        mka = mp.tile([32, 2, 512], bf16, tag="mka",
                                      name=f"mka{c}")
                        mkb = mp.tile([32, 2, 512], bf16, tag="mkb",
                                      name=f"mkb{c}")
                        # [kre*c, kre*s] and [kim*s, kim*c]
                        nc.vector.tensor_tensor(
                            mka[:, 0, :], kraw[0:32, :], t1_sb[0:32, ch], MULT)
                        nc.vector.tensor_tensor(
                            mka[:, 1, :], kraw[0:32, :], t2_sb[0:32, ch], MULT)
                        nc.vector.tensor_tensor(
                            mkb[:, 0, :], kraw[32:64, :], t2_sb[32:64, ch],
                            MULT)
                        nc.vector.tensor_tensor(
                            mkb[:, 1, :], kraw[32:64, :], t1_sb[32:64, ch],
                            MULT)
                        nc.vector.tensor_tensor(
                            kt_sb[0:32, ch], mka[:, 0, :], mkb[:, 0, :], SUB)
                        nc.vector.tensor_tensor(
                            kt_sb[32:64, ch], mka[:, 1, :], mkb[:, 1, :], ADD)
                        # dup K rows for the row-tiled head pair
                        nc.sync.dma_start(kt_sb[64:128, ch], kt_sb[0:64, ch])
                        for pb in range(2):
                            mqa = mp.tile([64, 2, 512], bf16, tag="mqa",
                                          name=f"mqa{c}_{pb}")
                            mqb = mp.tile([64, 2, 512], bf16, tag="mqb",
                                          name=f"mqb{c}_{pb}")
                            # re products: [ABre*c, ABre*s]
                            nc.vector.tensor_tensor(
                                mqa[:, 0, :], qraw[0:64, pb, :],
                                t1_sb[0:64, ch], MULT)
                            nc.vector.tensor_tensor(
                                mqa[:, 1, :], qraw[0:64, pb, :],
                                t2_sb[0:64, ch], MULT)
                            # im products: [ABim*s, ABim*c]
                            nc.vector.tensor_tensor(
                                mqb[:, 0, :], qraw[64:128, pb, :],
                                t1_sb[64:128, ch], MULT)
                            nc.vector.tensor_tensor(
                                mqb[:, 1, :], qraw[64:128, pb, :],
                                t2_sb[64:128, ch], MULT)
                            nc.vector.tensor_tensor(
                                qt_sb[0:32, pb, ch],
                                mqa[0:32, 0, :], mqb[0:32, 0, :], SUB)
                            nc.vector.tensor_tensor(
                                qt_sb[64:96, pb, ch],
                                mqa[32:64, 0, :], mqb[32:64, 0, :], SUB)
                            nc.vector.tensor_tensor(
                                qt_sb[32:64, pb, ch],
                                mqa[0:32, 1, :], mqb[0:32, 1, :], ADD)
                            nc.vector.tensor_tensor(
                                qt_sb[96:128, pb, ch],
                                mqa[32:64, 1, :], mqb[32:64, 1, :], ADD)

                    def vt_unit(c):
                        vps = ps_sA.tile([P, 4, 64], bf16, tag="s_ps",
                                         name=f"vps{c}")
                        for j in range(4):
                            nc.tensor.matmul(
                                vps[:, j, :], vt_sb[:, ts(4 * c + j, P)],
                                ident[:], start=True, stop=True,
                                is_transpose=True)
                        nc.vector.tensor_copy(
                            vn_sb[:, ds(4 * c, 4), 0:64], vps[:, :, :])

                    def stripe_units(s, ps_s, ps_o, pbs=(0, 1)):
                        qs = 512 * s
                        nkb = 4 * (s + 1)
                        units = []
                        for pb in pbs:
                            st = {}

                            def mk_unit(pb=pb, kb=0, st=st):
                                def unit():
                                    if "ops" not in st:
                                        st["ops"] = ps_o.tile(
                                            [P, 2, 512], f32, tag="o_ps",
                                            name=f"ops{s}_{pb}")
                                    ops = st["ops"]
                                    c0 = max(0, kb - 4 * s) * P
                                    w = 512 - c0
                                    diag = kb >= 4 * s
                                    sps = ps_s.tile([P, 2, 512], f32,
                                                    tag="s_ps",
                                                    name=f"sps{s}_{pb}_{kb}")
                                    for hh in range(2):
                                        nc.tensor.matmul(
                                            sps[:, hh, c0:512],
                                            kt_sb[ds(64 * hh, 64), ts(kb, P)],
                                            qt_sb[ds(64 * hh, 64), pb,
                                                  ds(qs + c0, w)],
                                            start=True, stop=not diag)
                                    if diag:
                                        for hh in range(2):
                                            nc.tensor.matmul(
                                                sps[:, hh, c0:c0 + P],
                                                negi[:], ustr[:],
                                                start=False, stop=True)
                                    et = etp.tile([P, 2, 512], bf16,
                                                  tag="e_t")
                                    if c0 == 0:
                                        nc.scalar.activation(
                                            et[:, :, :].rearrange(
                                                "p a b -> p (a b)"),
                                            sps[:, :, :].rearrange(
                                                "p a b -> p (a b)"),
                                            EXP, scale=0.125)
                                    else:
                                        nc.scalar.activation(
                                            et[:, :, c0:512],
                                            sps[:, :, c0:512],
                                            EXP, scale=0.125)
                                    for hh in range(2):
                                        nc.tensor.matmul(
                                            ops[:, hh, c0:512],
                                            vn_sb[:, kb, :],
                                            et[:, hh, ds(c0, w)],
                                            start=(kb == 0),
                                            stop=(kb == nkb - 1))
                                    if kb == nkb - 1:
                                        # split the last stripe's norm into
                                        # q-halves so the tail wo can start
                                        # after the first half
                                        nh = 2 if (s == 3 and pb == 1) else 1
                                        wq_ = 512 // nh
                                        for qh in range(nh):
                                            qsl = ds(qh * wq_, wq_)
                                            dn = rbp.tile(
                                                [64, 2, wq_], f32, tag="dn",
                                                name=f"dn{s}_{pb}_{qh}")
                                            rb = rbp.tile(
                                                [64, 2, wq_], f32, tag="rb",
                                                name=f"rb{s}_{pb}_{qh}")
                                            # custom-DVE recip needs SBUF in
                                            nc.vector.tensor_copy(
                                                dn[:, :, :],
                                                ops[64:128, :, qsl])
                                            nc.vector.reciprocal_approx_fast(
                                                rb[:, :, :].rearrange(
                                                    "p a b -> p (a b)"),
                                                dn[:, :, :].rearrange(
                                                    "p a b -> p (a b)"))
                                            for hh in range(2):
                                                nc.vector.tensor_tensor(
                                                    ot_sb[ds(64 * hh, 64), pb,
                                                          ds(qs + qh * wq_,
                                                             wq_)],
                                                    ops[0:64, hh, qsl],
                                                    rb[:, hh, :], MULT)
                                return unit

                            for kb in range(nkb):
                                units.append(mk_unit(pb=pb, kb=kb, st=st))
                        return units

                    # warm the PE while the first x bytes stream in: HAM
                    # un-throttles after ~3.4us of sustained matmul activity
                    # and re-throttles after ~3.4us idle, so keep it busy
                    # until the real stream begins (~20us in).
                    junk_n = [0]

                    def junk(k):
                        # junk matmuls keep HAM un-throttled across head
                        # gaps where no real PE work is ready yet
                        jt = ps_sA.tile([64, 512], f32, tag="s_ps",
                                        name=f"junk{junk_n[0]}")
                        junk_n[0] += 1
                        for _ in range(k):
                            nc.tensor.matmul(
                                jt[:, :], wsrc[:, 0:64], wsrc[:, :],
                                start=True, stop=True)

                    junk(24)
                    # chunk 0 (two position-halves for early start,
                    # pass-major so each pass starts as its input lands)
                    kraw0, qraw0 = proj_chunk(0, xts[0], pass_major=True)
                    xts.append(x_chunk_dma(2))
                    rope_chunk(0, kraw0, qraw0)
                    vt_unit(0)
                    # stripe 0 runs directly: it covers the xt1 DMA wait
                    # (all of its inputs come from chunk 0)
                    for u in stripe_units(0, ps_sA, ps_oA):
                        u()
                    # chunk 1 (stripe 1 needs rope1 - no drip yet)
                    kraw1, qraw1 = proj_chunk(1, xts[1])
                    xts.append(x_chunk_dma(3))
                    rope_chunk(1, kraw1, qraw1)
                    vt_unit(1)
                    # chunk 2, stripe 1 dripped in
                    drip += stripe_units(1, ps_sA, ps_oA)
                    kraw2, qraw2 = proj_chunk(2, xts[2], rate=5)
                    rope_chunk(2, kraw2, qraw2)
                    vt_unit(2)
                    # chunk 3, stripe-2 pb0 dripped in (pb1 goes to phase B
                    # so the A->B psum-pool barrier comes sooner)
                    drip += stripe_units(2, ps_sA, ps_oA, pbs=(0,))
                    kraw3, qraw3 = proj_chunk(3, xts[3], rate=8)
                    # wo weights (queued after x on the SWDGE ring)
                    in_cast_dma(
                        wo_sb[:, :, :],
                        wo_p[:, :].rearrange("(c p) d -> p c d", p=P))
                    rope_chunk(3, kraw3, qraw3)
                    vt_unit(3)
                    while drip:
                        pop_drip()

                # ---------- phase B: stripes 2-3 + wo ----------
                with (
                    tc.tile_pool(name="ps_sB", bufs=2, space="PSUM") as ps_sB,
                    tc.tile_pool(name="ps_oB", bufs=1, space="PSUM") as ps_oB,
                    tc.tile_pool(name="ps_w", bufs=2, space="PSUM") as ps_w,
                ):
                    ob_state = {}
                    evac_flip = [0]

                    def wo_unit(lq, n, hf, evac_act=None):
                        def unit():
                            key = (lq, n)
                            if key not in ob_state:
                                ob_state[key] = obp.tile(
                                    [P, 1024], bf16, tag="o_sb",
                                    name=f"ob{lq}_{n}")
                            ob = ob_state[key]
                            wps = ps_w.tile([P, 512], f32, tag="w_ps",
                                            name=f"wps{lq}_{n}_{hf}")
                            for t in range(2):
                                nc.tensor.matmul(
                                    wps[:, :], ot_sb[:, t, ts(lq, P)],
                                    wo_sb[:, t, ds(n * 1024 + hf * 512, 512)],
                                    start=(t == 0), stop=(t == 1))
                            use_act = evac_act
                            if use_act is None:
                                use_act = False
                            if use_act:
                                nc.scalar.copy(
                                    ob[:, ds(512 * hf, 512)], wps[:, :])
                            else:
                                nc.vector.tensor_copy(
                                    ob[:, ds(512 * hf, 512)], wps[:, :])
                            # stream each half out as soon as it lands
                            deng = nc.sync if (lq + n) % 2 == 0 \
                                else nc.gpsimd
                            deng.dma_start(
                                part[ts(lq, P), ds(n * 1024 + hf * 512, 512)],
                                ob[:, ds(512 * hf, 512)])
                        return unit

                    drip += [wo_unit(lq, n, hf)
                             for lq in range(0, 12)
                             for n in range(2) for hf in range(2)]
                    for u in stripe_units(2, ps_sB, ps_oB, pbs=(1,)):
                        u()
                        pop_drip(1)
                    for u in stripe_units(3, ps_sB, ps_oB):
                        u()
                        pop_drip(2)
                    while drip:
                        pop_drip()
                    for lq in range(12, 16):
                        for n in range(2):
                            for hf in range(2):
                                wo_unit(lq, n, hf, evac_act=(hf == 0))()

    nc.compile()
    return nc


_NC_CACHE = None


def _get_nc():
    global _NC_CACHE
    if _NC_CACHE is None:
        _NC_CACHE = _build_nc()
    return _NC_CACHE


def _shard_inputs(x, wq, wk, wv, wo, freqs_cos, freqs_sin, mask):
    """Host-side shard prep: pure layout/indexing transforms, no arithmetic."""
    f = np.float32
    perm = np.empty(64, np.int64)
    perm[:32] = 2 * np.arange(32)
    perm[32:] = 2 * np.arange(32) + 1

    xr = np.asarray(x, f).reshape(L, D)
    x_b = np.ascontiguousarray(
        xr.reshape(8, 256, KT, P).transpose(0, 3, 2, 1)
        .reshape(8 * P, KT * 256))
    cosT = np.ascontiguousarray(np.asarray(freqs_cos, f).T)
    sinT = np.ascontiguousarray(np.asarray(freqs_sin, f).T)
    cos2 = np.ascontiguousarray(np.concatenate([cosT, cosT], 0))
    sin2 = np.ascontiguousarray(np.concatenate([sinT, sinT], 0))

    wq = np.asarray(wq, f)
    wk = np.asarray(wk, f)
    wv = np.asarray(wv, f)
    wo = np.asarray(wo, f)

    in_maps = []
    for i in range(NCORES):
        wq_i = wq[:, 4 * i * 64:(4 * i + 4) * 64]
        cols = []
        for pb in range(2):
            A = wq_i[:, (2 * pb) * 64:(2 * pb + 1) * 64][:, perm]
            Bc = wq_i[:, (2 * pb + 1) * 64:(2 * pb + 2) * 64][:, perm]
            cols.append(np.concatenate(
                [A[:, :32], Bc[:, :32], A[:, 32:], Bc[:, 32:]], 1))
        wq_p = np.concatenate(cols, 1)
        # device layout: [p, t*n] with row 128t+p -> partition p, chunk t
        wq_p = np.ascontiguousarray(
            wq_p.reshape(KT, P, 256).transpose(1, 0, 2).reshape(P, KT * 256))
        wk_p = wk[:, i * 64:(i + 1) * 64][:, perm]
        wv_i = wv[:, i * 64:(i + 1) * 64]
        wkv = np.concatenate([wk_p, wv_i], 1)
        wkv = np.ascontiguousarray(
            wkv.reshape(KT, P, 128).transpose(1, 0, 2).reshape(P, KT * 128))
        wo_i = np.ascontiguousarray(wo[4 * i * 64:(4 * i + 4) * 64, :])
        in_maps.append({
            "x_b": x_b, "wq_p": wq_p, "wkv": wkv, "wo_p": wo_i,
            "cos2": cos2, "sin2": sin2,
        })
    return in_maps


_last_results = None


def kernel(x, wq, wk, wv, wo, freqs_cos, freqs_sin, mask):
    global _last_results
    from concourse.bass_utils import run_bass_kernel_spmd

    nc = _get_nc()
    in_maps = _shard_inputs(x, wq, wk, wv, wo, freqs_cos, freqs_sin, mask)
    res = run_bass_kernel_spmd(nc, in_maps, core_ids=list(range(NCORES)))
    _last_results = res
    out = np.zeros((L, D), np.float32)
    for i in range(NCORES):
        out += np.asarray(res.results[i]["part"]).astype(np.float32)
    return out.reshape(B, L, D)
